# revision 14
# baseline (speedup 1.0000x reference)
"""Trainium2 Bass kernel for pre-LN multi-head attention.

Reference computation (B=2, N=2048, D=1024, H=16, DH=64):
    xn = LayerNorm(x) * g + b
    q = xn @ Wq ; k, v = split(xn @ Wkv)
    out = softmax(q k^T / sqrt(DH)) v  (per head)
    return out @ Wout

Sharding: core c handles batch b = c // 4 and heads 4*(c%4) .. 4*(c%4)+3.
Each core computes a partial output (its 4 heads' slice of the out
projection); the host sums the 4 partials per batch.

Host-side folding: ln_g is folded into the weight rows, ln_b becomes a
per-output-column bias (cq/ck/cv); the attention scale DH^-0.5 is folded
into Wq. On-chip LN is then just (x - mean) * rsqrt(var + eps).

Design notes (v2):
- All transposes ride the DMA crossbar (dma_start_transpose, 14ns per
  16x128 tile): xn^T (zT) and the v [keys, dim] layout are produced by
  SBUF->SBUF transpose DMAs, so the PE does zero transpose work and
  stage 1 needs no PSUM at all.
- Every matmul in the kernel is a full 128-partition contraction with
  tile_size (128,128): the dots use a zero-padded kT (kTz) where the
  other head's 64 contraction rows are zeroed, so q/k of two heads can
  stay stacked in one partition range without PE array-mode switches
  (mode reconfig drains the PE).
- Softmax exp is split across two engines by query column: q-cols
  0-1023 get exact Exp on ScalarE; q-cols 1024-2047 get a one-pass
  Schraudolph fast-exp on VectorE (z = round(d*128/ln2 + magic) as
  int16, bitcast to bf16 = 2^(d/ln2) with ~2% column-consistent noise
  that the softmax normalization largely cancels). Per-(q,head) all
  weights come from the same engine, so the rowsum normalization
  cancels the approximation's common mode exactly.
- The softmax rowsum comes from a ones-column appended to V (M=65 costs
  the same PE time as M=64); the reciprocal is computed on 64 lanes via
  a DRAM round-trip and partition-broadcast back (SBUF stride-0
  partition APs are rejected by the runtime).
- PSUM budget (8 banks): 2 dots tiles [128,1024] (4 banks) + 2 EV
  accumulators [65,1024] (4 banks) live during attention; QKV/out-proj
  phases use their own rotating [128,1024] tiles.
- Attention math is bf16; LN runs fp32 on DVE; PSUM accumulation fp32.
"""

from contextlib import ExitStack

import numpy as np

import concourse.bass as bass
import concourse.mybir as mybir
import concourse.tile as tile
from concourse import bacc
from concourse.bass_utils import run_bass_kernel_spmd

B, N, D = 2, 2048, 1024
H, DH = 16, 64
EPS = 1e-5
N_CORES = 8
HPC = 4          # heads per core
CW = HPC * DH    # 256 columns of q/k/v per core

f32 = mybir.dt.float32
bf16 = mybir.dt.bfloat16
i16 = mybir.dt.int16
AF = mybir.ActivationFunctionType
ALU = mybir.AluOpType

# Schraudolph fast-exp in bf16-bit domain: round(d*128/ln2 + 127*128 - c)
# reinterpreted as bf16 equals 2^(d/ln2) * (1 + eps(frac)), |eps| <= 3%.
FE_S1 = 128.0 / float(np.log(2.0))
FE_S2 = 127.0 * 128.0 - 0.0579 * 128.0

TRACE = False
LAST_RESULT = None
_compiled = None


def _build():
    nc = bacc.Bacc("TRN2", target_bir_lowering=False, debug=False,
                   num_devices=N_CORES)

    x_d = nc.dram_tensor("x", [N, D], f32, kind="ExternalInput")
    wq_d = nc.dram_tensor("wq", [D, CW], f32, kind="ExternalInput")
    wk_d = nc.dram_tensor("wk", [D, CW], f32, kind="ExternalInput")
    wv_d = nc.dram_tensor("wv", [D, CW], f32, kind="ExternalInput")
    wo_d = nc.dram_tensor("wo", [CW, D], f32, kind="ExternalInput")
    cq_d = nc.dram_tensor("cq", [CW], f32, kind="ExternalInput")
    ck_d = nc.dram_tensor("ck", [CW], f32, kind="ExternalInput")
    cv_d = nc.dram_tensor("cv", [CW], f32, kind="ExternalInput")
    out_d = nc.dram_tensor("out", [N, D], f32, kind="ExternalOutput")
    rec_d = nc.dram_tensor("rec_scratch", [8, N // 2], f32)
    sum_d = nc.dram_tensor("sum_scratch", [8, N // 2], f32)

    with tile.TileContext(nc) as tc, ExitStack() as ctx:
        consts = ctx.enter_context(tc.tile_pool(name="consts", bufs=1))
        eps_t = consts.tile([128, 1], f32)
        nc.vector.memset(eps_t, EPS)

        # persistent attention operands
        qkp = ctx.enter_context(tc.tile_pool(name="qkp", bufs=1))
        qT = qkp.tile([128, 2, N], bf16)
        # kTz: per head, the other head's 64 contraction rows are zero so
        # dots can contract all 128 partitions (no PE mode switches).
        kTz = qkp.tile([128, HPC, N], bf16)
        vA = qkp.tile([128, 16, HPC * (DH + 1)], bf16)
        vA4 = vA.rearrange("p k (h c) -> p k h c", h=HPC)
        outT = qkp.tile([128, 2, N], bf16)

        # x prefetch: rotating 4-deep ring on the scalar DMA queue (sync
        # stays free for the stage-1 transposes, gpsimd for weight loads).
        xpre_cm = tc.tile_pool(name="xpre", bufs=4)
        xpre = xpre_cm.__enter__()
        x_r = x_d.rearrange("(t u p) d -> t p u d", p=128, u=2)
        x_tiles = []
        for rt in range(8):
            xt = xpre.tile([128, 2, D], f32, tag="xt", name=f"xt{rt}")
            nc.scalar.dma_start(out=xt, in_=x_r[rt])
            x_tiles.append(xt)

        cq_t = consts.tile([128, 2], f32)
        nc.sync.dma_start(out=cq_t, in_=cq_d.rearrange("(j p) -> p j", p=128))
        ck_t = consts.tile([128, 2], f32)
        nc.sync.dma_start(out=ck_t, in_=ck_d.rearrange("(j p) -> p j", p=128))
        cv_t = consts.tile([128, 2], f32)
        nc.sync.dma_start(out=cv_t, in_=cv_d.rearrange("(j p) -> p j", p=128))

        # weights: DMA fp32 staging + bf16 casts + the zero/one memsets all
        # ride gpsimd (engine and queue otherwise idle); interleaved so wq
        # is ready first and kTz zeros land before the first k eviction.
        wstage_cm = tc.tile_pool(name="wstage", bufs=1)
        wstage = wstage_cm.__enter__()
        wq_t = consts.tile([128, 8, CW], bf16)
        wk_t = consts.tile([128, 8, CW], bf16)
        wv_t = consts.tile([128, 8, CW], bf16)
        wo_t = consts.tile([128, 2, D], bf16)
        for i, (dram, dst, spec) in enumerate(
                ((wq_d, wq_t, "(t p) m -> p t m"),
                 (wk_d, wk_t, "(t p) m -> p t m"),
                 (wv_d, wv_t, "(t p) m -> p t m"),
                 (wo_d, wo_t, "(j p) d -> p j d"))):
            src = dram.rearrange(spec, p=128)
            stg = wstage.tile(list(src.shape), f32, tag="wstg", name="wstg")
            nc.gpsimd.dma_start(out=stg, in_=src)
            nc.gpsimd.tensor_copy(out=dst, in_=stg)
            if i == 0:
                nc.gpsimd.memset(kTz, 0.0)
            elif i == 1:
                nc.gpsimd.memset(vA4[:, :, :, DH:DH + 1], 1.0)

        # z^T in bf16: [chan(128 per tile) x 8 chan-tiles x N rows]
        zTp_cm = tc.tile_pool(name="zTp", bufs=1)
        zTp = zTp_cm.__enter__()
        zT = zTp.tile([128, 8, N], bf16)

        # ---- stage 1: LayerNorm -> zt (bf16) -> DMA-crossbar transpose ----
        vTp_cm = tc.tile_pool(name="vTp", bufs=1)
        vTp = vTp_cm.__enter__()
        vT = vTp.tile([128, 2, N], bf16)

        zp_cm = tc.tile_pool(name="zp", bufs=2)
        zp = zp_cm.__enter__()
        stp_cm = tc.tile_pool(name="stp", bufs=4)
        stp = stp_cm.__enter__()
        ps2_cm = tc.tile_pool(name="ps2", bufs=1, space="PSUM")
        ps2 = ps2_cm.__enter__()

        for rt in range(8):
            xt = x_tiles[rt]
            zt = zp.tile([128, 2, D], bf16, tag="zt", name=f"zt{rt}")
            scales = []
            for u in range(2):
                st = stp.tile([128, 2, 6], f32, tag="st", name=f"st{rt}_{u}")
                nc.vector.bn_stats(out=st[:, 0], in_=xt[:, u, 0:512])
                nc.vector.bn_stats(out=st[:, 1], in_=xt[:, u, 512:1024])
                mv = stp.tile([128, 2], f32, tag="mv", name=f"mv{rt}_{u}")
                nc.vector.bn_aggr(out=mv, in_=st)
                rstd = stp.tile([128, 1], f32, tag="rstd",
                                name=f"rstd{rt}_{u}")
                nc.scalar.activation(out=rstd, in_=mv[:, 1:2], func=AF.Sqrt,
                                     bias=eps_t, scale=1.0)
                nc.vector.reciprocal(out=rstd, in_=rstd)
                nmr = stp.tile([128, 1], f32, tag="nmr", name=f"nmr{rt}_{u}")
                nc.vector.tensor_scalar(out=nmr, in0=mv[:, 0:1],
                                        scalar1=rstd, scalar2=-1.0,
                                        op0=ALU.mult, op1=ALU.mult)
                scales.append((rstd, nmr))
            for u in range(2):
                rstd, nmr = scales[u]
                nc.scalar.activation(out=zt[:, u, :], in_=xt[:, u, :],
                                     func=AF.Identity, bias=nmr, scale=rstd)
                r0 = rt * 256 + u * 128
                nc.sync.dma_start(out=zT[:, :, r0:r0 + 128], in_=zt[:, u, :],
                                  transpose=True)

        # ---- stage 2: QKV projections (all transposed layout, 1024-wide) --
        for chk in range(2):
            ns = slice(chk * 1024, (chk + 1) * 1024)
            for pi, (w_t, c_t, dest) in enumerate(((wq_t, cq_t, qT),
                                                   (wk_t, ck_t, None),
                                                   (wv_t, cv_t, vT))):
                for j in range(2):
                    pq = ps2.tile([128, 1024], f32, tag="pq", bufs=3,
                                  name=f"pq{chk}_{pi}_{j}")
                    for t in range(8):
                        lhs = w_t[:, t, j * 128:(j + 1) * 128]
                        for half in range(2):
                            hs = slice(half * 512, (half + 1) * 512)
                            nc.tensor.matmul(
                                pq[:, hs], lhs,
                                zT[:, t, chk * 1024 + half * 512:
                                   chk * 1024 + half * 512 + 512],
                                start=(t == 0), stop=(t == 7))
                    if pi == 0:
                        nc.scalar.activation(out=dest[:, j, ns], in_=pq,
                                             func=AF.Identity,
                                             bias=c_t[:, j:j + 1], scale=1.0)
                    elif pi == 1:
                        # k lands in the zero-padded kTz halves
                        for par in range(2):
                            ps = slice(par * 64, (par + 1) * 64)
                            nc.vector.tensor_scalar_add(
                                out=kTz[ps, 2 * j + par, ns], in0=pq[ps, :],
                                scalar1=c_t[ps, j:j + 1])
                    else:
                        eng = nc.scalar if j == 0 else nc.vector
                        if j == 0:
                            nc.scalar.activation(out=dest[:, j, ns], in_=pq,
                                                 func=AF.Identity,
                                                 bias=c_t[:, j:j + 1],
                                                 scale=1.0)
                        else:
                            nc.vector.tensor_scalar_add(
                                out=dest[:, j, ns], in0=pq,
                                scalar1=c_t[:, j:j + 1])

        # v: [vdim, n] -> [keys, kt, head, dim] via DMA-crossbar transpose.
        # The XBAR needs a 16B-aligned contiguous destination, so transpose
        # into a tmp tile and strided-copy into the 65-col augmented layout.
        for j in range(2):
            vtmp = vTp.tile([128, 16, 128], bf16, tag="vtmp", bufs=2,
                            name=f"vtmp{j}")
            nc.scalar.dma_start(out=vtmp, in_=vT[:, j, :], transpose=True)
            for hp in range(2):
                nc.scalar.dma_start(
                    out=vA4[:, :, 2 * j + hp, 0:DH],
                    in_=vtmp[:, :, 64 * hp:64 * hp + 64])

        ps2_cm.__exit__(None, None, None)
        stp_cm.__exit__(None, None, None)
        zp_cm.__exit__(None, None, None)
        vTp_cm.__exit__(None, None, None)
        zTp_cm.__exit__(None, None, None)
        wstage_cm.__exit__(None, None, None)
        xpre_cm.__exit__(None, None, None)

        # ---- stage 3: attention ----
        # Per head: dots (q-halves into 2 PSUM tiles), exp on ACT (cols
        # 0-1023, exact) and DVE (cols 1024-2047, fast-exp), AV accumulated
        # over the 16 key tiles into [65, 1024] PSUM accumulators.
        with tc.tile_pool(name="Ep", bufs=1) as Ep, \
             tc.tile_pool(name="rp", bufs=1) as rp, \
             tc.tile_pool(name="psD", bufs=1, space="PSUM") as psD, \
             tc.tile_pool(name="psU", bufs=1, space="PSUM") as psU:
            for h in range(HPC):
                j, p0 = h // 2, 64 * (h % 2)
                pU0 = psU.tile([DH + 1, 1024], f32, tag="pU0", name=f"pU0_{h}")
                pU1 = psU.tile([DH + 1, 1024], f32, tag="pU1", name=f"pU1_{h}")
                vh = vA[:, :, h * (DH + 1):(h + 1) * (DH + 1)]
                for kt in range(16):
                    ksl = slice(kt * 128, (kt + 1) * 128)
                    pD0 = psD.tile([128, 1024], f32, tag="pD0",
                                   name=f"pD0_{h}_{kt}")
                    pD1 = psD.tile([128, 1024], f32, tag="pD1",
                                   name=f"pD1_{h}_{kt}")
                    lhs = kTz[:, h, ksl]
                    for half in range(2):
                        hs = slice(half * 512, (half + 1) * 512)
                        nc.tensor.matmul(pD0[:, hs], lhs,
                                         qT[:, j, half * 512:half * 512 + 512],
                                         start=True, stop=True)
                        nc.tensor.matmul(pD1[:, hs], lhs,
                                         qT[:, j, 1024 + half * 512:
                                            1024 + half * 512 + 512],
                                         start=True, stop=True)
                    Et0 = Ep.tile([128, 1024], bf16, tag="Et0", bufs=2,
                                  name=f"Et0_{h}_{kt}")
                    nc.scalar.activation(out=Et0, in_=pD0, func=AF.Exp,
                                         bias=0.0, scale=1.0)
                    Et1 = Ep.tile([128, 1024], i16, tag="Et1", bufs=2,
                                  name=f"Et1_{h}_{kt}")
                    nc.vector.tensor_scalar(out=Et1, in0=pD1,
                                            scalar1=FE_S1, scalar2=FE_S2,
                                            op0=ALU.mult, op1=ALU.add)
                    Et1b = Et1.bitcast(bf16)
                    for half in range(2):
                        hs = slice(half * 512, (half + 1) * 512)
                        nc.tensor.matmul(pU0[:, hs], vh[:, kt, :], Et0[:, hs],
                                         start=(kt == 0), stop=(kt == 15))
                        nc.tensor.matmul(pU1[:, hs], vh[:, kt, :], Et1b[:, hs],
                                         start=(kt == 0), stop=(kt == 15))
                # normalization: rowsum (row 64) -> DRAM -> [64,16] lanes ->
                # reciprocal -> DRAM -> partition-broadcast -> multiply
                for qh, pU in ((0, pU0), (1, pU1)):
                    slot = sum_d[h * 2 + qh]
                    uS = rp.tile([1, 1024], f32, tag="uS", bufs=2,
                                 name=f"uS_{h}_{qh}")
                    nc.scalar.copy(out=uS, in_=pU[DH:DH + 1, :])
                    nc.sync.dma_start(out=slot, in_=uS)
                    r8 = rp.tile([64, 16], f32, tag="r8", bufs=2,
                                 name=f"r8_{h}_{qh}")
                    nc.sync.dma_start(
                        out=r8, in_=slot.rearrange("(p e) -> p e", p=64))
                    nc.vector.reciprocal(out=r8, in_=r8)
                    rslot = rec_d[h * 2 + qh]
                    nc.sync.dma_start(out=rslot, in_=r8)
                    recB = rp.tile([64, 1024], f32, tag="recB", bufs=2,
                                   name=f"recB_{h}_{qh}")
                    rbc = bass.AP(tensor=rslot.tensor, offset=rslot.offset,
                                  ap=[[0, 64]] + list(rslot.ap))
                    nc.gpsimd.dma_start(out=recB, in_=rbc)
                    nc.vector.tensor_mul(
                        out=outT[p0:p0 + 64, j, qh * 1024:(qh + 1) * 1024],
                        in0=pU[0:DH, :], in1=recB)

        # ---- stage 4: output projection ----
        with tc.tile_pool(name="osb", bufs=4) as osb, \
             tc.tile_pool(name="psO", bufs=1, space="PSUM") as psO:
            out_r = out_d.rearrange("(m p) d -> m p d", p=128)
            for m in range(16):
                pO = psO.tile([128, 1024], f32, tag="pO", bufs=2,
                              name=f"pO{m}")
                for j in range(2):
                    lhs = outT[:, j, m * 128:(m + 1) * 128]
                    for half in range(2):
                        hs = slice(half * 512, (half + 1) * 512)
                        nc.tensor.matmul(pO[:, hs], lhs, wo_t[:, j, hs],
                                         start=(j == 0), stop=(j == 1))
                ot = osb.tile([128, 1024], f32, tag="ot", name=f"ot{m}")
                if m % 2 == 0:
                    nc.scalar.activation(out=ot, in_=pO, func=AF.Identity,
                                         bias=0.0, scale=1.0)
                else:
                    nc.vector.tensor_copy(out=ot, in_=pO)
                eng = nc.sync if m % 2 == 0 else nc.scalar
                eng.dma_start(out=out_r[m], in_=ot)

    nc.compile()
    return nc


def make_in_maps(x, ln_g, ln_b, Wq, Wkv, Wout):
    x = np.asarray(x, np.float32)
    ln_g = np.asarray(ln_g, np.float32)
    ln_b = np.asarray(ln_b, np.float32)
    Wq = np.asarray(Wq, np.float32)
    Wkv = np.asarray(Wkv, np.float32)
    Wout = np.asarray(Wout, np.float32)

    scale = DH ** -0.5
    Wq_f = (ln_g[:, None] * Wq) * scale
    cq_f = (ln_b @ Wq) * scale
    Wk_f = ln_g[:, None] * Wkv[:, :D]
    ck_f = ln_b @ Wkv[:, :D]
    Wv_f = ln_g[:, None] * Wkv[:, D:]
    cv_f = ln_b @ Wkv[:, D:]

    in_maps = []
    for c in range(N_CORES):
        cols = slice((c % 4) * CW, (c % 4 + 1) * CW)
        in_maps.append({
            "x": np.ascontiguousarray(x[c // 4]),
            "wq": np.ascontiguousarray(Wq_f[:, cols]),
            "wk": np.ascontiguousarray(Wk_f[:, cols]),
            "wv": np.ascontiguousarray(Wv_f[:, cols]),
            "wo": np.ascontiguousarray(Wout[cols, :]),
            "cq": np.ascontiguousarray(cq_f[cols]),
            "ck": np.ascontiguousarray(ck_f[cols]),
            "cv": np.ascontiguousarray(cv_f[cols]),
        })
    return in_maps


def kernel(x, ln_g, ln_b, Wq, Wkv, Wout):
    global _compiled, LAST_RESULT
    if _compiled is None:
        _compiled = _build()
    nc = _compiled

    in_maps = make_in_maps(x, ln_g, ln_b, Wq, Wkv, Wout)
    res = run_bass_kernel_spmd(nc, in_maps, list(range(N_CORES)), trace=TRACE)
    LAST_RESULT = res

    out = np.zeros((B, N, D), np.float32)
    for c in range(N_CORES):
        out[c // 4] += res.results[c]["out"]
    return out


# revision 17
# speedup vs baseline: 1.2818x; 1.2818x over previous
"""Trainium2 Bass kernel for pre-LN multi-head attention.

Reference computation (B=2, N=2048, D=1024, H=16, DH=64):
    xn = LayerNorm(x) * g + b
    q = xn @ Wq ; k, v = split(xn @ Wkv)
    out = softmax(q k^T / sqrt(DH)) v  (per head)
    return out @ Wout

Sharding: core c handles batch b = c // 4 and heads 4*(c%4) .. 4*(c%4)+3.
Each core computes a partial output (its 4 heads' slice of the out
projection); the host sums the 4 partials per batch.

Host-side folding: ln_g is folded into the weight rows, ln_b becomes a
per-output-column bias (cq/ck/cv); the attention scale DH^-0.5 is folded
into Wq. On-chip LN is then just (x - mean) * rsqrt(var + eps).

Design notes (v2):
- All transposes ride the DMA crossbar (dma_start_transpose, 14ns per
  16x128 tile): xn^T (zT) and the v [keys, dim] layout are produced by
  SBUF->SBUF transpose DMAs, so the PE does zero transpose work and
  stage 1 needs no PSUM at all.
- Every matmul in the kernel is a full 128-partition contraction with
  tile_size (128,128): the dots use a zero-padded kT (kTz) where the
  other head's 64 contraction rows are zeroed, so q/k of two heads can
  stay stacked in one partition range without PE array-mode switches
  (mode reconfig drains the PE).
- Softmax exp is split across two engines by query column: q-cols
  0-1023 get exact Exp on ScalarE; q-cols 1024-2047 get a one-pass
  Schraudolph fast-exp on VectorE (z = round(d*128/ln2 + magic) as
  int16, bitcast to bf16 = 2^(d/ln2) with ~2% column-consistent noise
  that the softmax normalization largely cancels). Per-(q,head) all
  weights come from the same engine, so the rowsum normalization
  cancels the approximation's common mode exactly.
- The softmax rowsum comes from a ones-column appended to V (M=65 costs
  the same PE time as M=64); the reciprocal is computed on 64 lanes via
  a DRAM round-trip and partition-broadcast back (SBUF stride-0
  partition APs are rejected by the runtime).
- PSUM budget (8 banks): 2 dots tiles [128,1024] (4 banks) + 2 EV
  accumulators [65,1024] (4 banks) live during attention; QKV/out-proj
  phases use their own rotating [128,1024] tiles.
- Attention math is bf16; LN runs fp32 on DVE; PSUM accumulation fp32.
"""

from contextlib import ExitStack

import numpy as np

import concourse.bass as bass
import concourse.mybir as mybir
import concourse.tile as tile
from concourse import bacc
from concourse.bass_utils import run_bass_kernel_spmd

B, N, D = 2, 2048, 1024
H, DH = 16, 64
EPS = 1e-5
N_CORES = 8
HPC = 4          # heads per core
CW = HPC * DH    # 256 columns of q/k/v per core

f32 = mybir.dt.float32
bf16 = mybir.dt.bfloat16
i16 = mybir.dt.int16
AF = mybir.ActivationFunctionType
ALU = mybir.AluOpType

# Schraudolph fast-exp in bf16-bit domain: round(d*128/ln2 + 127*128 - c)
# reinterpreted as bf16 equals 2^(d/ln2) * (1 + eps(frac)), |eps| <= 3%.
FE_S1 = 128.0 / float(np.log(2.0))
FE_S2 = 127.0 * 128.0 - 0.0579 * 128.0

TRACE = False
LAST_RESULT = None
_compiled = None


def _build():
    nc = bacc.Bacc("TRN2", target_bir_lowering=False, debug=False,
                   num_devices=N_CORES)

    x_d = nc.dram_tensor("x", [N, D], f32, kind="ExternalInput")
    wq_d = nc.dram_tensor("wq", [D, CW], f32, kind="ExternalInput")
    wk_d = nc.dram_tensor("wk", [D, CW], f32, kind="ExternalInput")
    wv_d = nc.dram_tensor("wv", [D, CW], f32, kind="ExternalInput")
    wo_d = nc.dram_tensor("wo", [CW, D], f32, kind="ExternalInput")
    cq_d = nc.dram_tensor("cq", [CW], f32, kind="ExternalInput")
    ck_d = nc.dram_tensor("ck", [CW], f32, kind="ExternalInput")
    cv_d = nc.dram_tensor("cv", [CW], f32, kind="ExternalInput")
    out_d = nc.dram_tensor("out", [N, D], f32, kind="ExternalOutput")
    rec_d = nc.dram_tensor("rec_scratch", [8, N // 2], f32)
    sum_d = nc.dram_tensor("sum_scratch", [8, N // 2], f32)

    with tile.TileContext(nc) as tc, ExitStack() as ctx:
        consts = ctx.enter_context(tc.tile_pool(name="consts", bufs=1))
        eps_t = consts.tile([128, 1], f32)
        nc.vector.memset(eps_t, EPS)

        # persistent attention operands
        qkp = ctx.enter_context(tc.tile_pool(name="qkp", bufs=1))
        qT = qkp.tile([128, 2, N], bf16)
        # kTz: per head, the other head's 64 contraction rows are zero so
        # dots can contract all 128 partitions (no PE mode switches).
        kTz = qkp.tile([128, HPC, N], bf16)
        vA = qkp.tile([128, 16, HPC * (DH + 1)], bf16)
        vA4 = vA.rearrange("p k (h c) -> p k h c", h=HPC)
        outT = qkp.tile([128, 2, N], bf16)

        # x prefetch: 6-deep ring on the gpsimd DMA queue (issuing DMAs on
        # the scalar queue steals ~0.8us of ACT per issue). The kTz/ones
        # memsets fill the queue gap while the ring waits on LN consumption.
        xpre_cm = tc.tile_pool(name="xpre", bufs=6)
        xpre = xpre_cm.__enter__()
        x_r = x_d.rearrange("(t u p) d -> t p u d", p=128, u=2)
        x_tiles = []
        for rt in range(8):
            xt = xpre.tile([128, 2, D], f32, tag="xt", name=f"xt{rt}")
            nc.gpsimd.dma_start(out=xt, in_=x_r[rt])
            x_tiles.append(xt)
            if rt == 5:
                nc.gpsimd.memset(kTz, 0.0)
                nc.gpsimd.memset(vA4[:, :, :, DH:DH + 1], 1.0)

        cq_t = consts.tile([128, 2], f32)
        nc.sync.dma_start(out=cq_t, in_=cq_d.rearrange("(j p) -> p j", p=128))
        ck_t = consts.tile([128, 2], f32)
        nc.sync.dma_start(out=ck_t, in_=ck_d.rearrange("(j p) -> p j", p=128))
        cv_t = consts.tile([128, 2], f32)
        nc.sync.dma_start(out=cv_t, in_=cv_d.rearrange("(j p) -> p j", p=128))

        # weights: fp32 staging DMAs on sync, bf16 casts split ACT/DVE;
        # emission is interleaved into the LN loop below so wq is ready
        # before the first projection but the zT transposes aren't delayed.
        wstage_cm = tc.tile_pool(name="wstage", bufs=2)
        wstage = wstage_cm.__enter__()
        wq_t = consts.tile([128, 8, CW], bf16)
        wk_t = consts.tile([128, 8, CW], bf16)
        wv_t = consts.tile([128, 8, CW], bf16)
        wo_t = consts.tile([128, 2, D], bf16)
        _wjobs = [(wq_d, wq_t, "(t p) m -> p t m", nc.scalar),
                  (wk_d, wk_t, "(t p) m -> p t m", nc.vector),
                  (wv_d, wv_t, "(t p) m -> p t m", nc.scalar),
                  (wo_d, wo_t, "(j p) d -> p j d", nc.vector)]

        def _stage_weight(i):
            dram, dst, spec, eng = _wjobs[i]
            src = dram.rearrange(spec, p=128)
            stg = wstage.tile(list(src.shape), f32, tag="wstg", name="wstg")
            nc.sync.dma_start(out=stg, in_=src)
            if eng is nc.scalar:
                nc.scalar.activation(out=dst, in_=stg, func=AF.Identity,
                                     bias=0.0, scale=1.0)
            else:
                nc.vector.tensor_copy(out=dst, in_=stg)

        # z^T in bf16: [chan(128 per tile) x 8 chan-tiles x N rows]
        zTp_cm = tc.tile_pool(name="zTp", bufs=1)
        zTp = zTp_cm.__enter__()
        zT = zTp.tile([128, 8, N], bf16)

        # ---- stage 1: LayerNorm -> zt (bf16) -> DMA-crossbar transpose ----
        vTp_cm = tc.tile_pool(name="vTp", bufs=1)
        vTp = vTp_cm.__enter__()
        vT = vTp.tile([128, 2, N], bf16)

        zp_cm = tc.tile_pool(name="zp", bufs=2)
        zp = zp_cm.__enter__()
        stp_cm = tc.tile_pool(name="stp", bufs=4)
        stp = stp_cm.__enter__()
        ps2_cm = tc.tile_pool(name="ps2", bufs=1, space="PSUM")
        ps2 = ps2_cm.__enter__()

        for rt in range(8):
            if rt in (0, 2, 4, 6):
                _stage_weight(rt // 2)
            xt = x_tiles[rt]
            zt = zp.tile([128, 2, D], bf16, tag="zt", name=f"zt{rt}")
            scales = []
            for u in range(2):
                st = stp.tile([128, 2, 6], f32, tag="st", name=f"st{rt}_{u}")
                nc.vector.bn_stats(out=st[:, 0], in_=xt[:, u, 0:512])
                nc.vector.bn_stats(out=st[:, 1], in_=xt[:, u, 512:1024])
                mv = stp.tile([128, 2], f32, tag="mv", name=f"mv{rt}_{u}")
                nc.vector.bn_aggr(out=mv, in_=st)
                rstd = stp.tile([128, 1], f32, tag="rstd",
                                name=f"rstd{rt}_{u}")
                nc.scalar.activation(out=rstd, in_=mv[:, 1:2], func=AF.Sqrt,
                                     bias=eps_t, scale=1.0)
                nc.vector.reciprocal(out=rstd, in_=rstd)
                nmr = stp.tile([128, 1], f32, tag="nmr", name=f"nmr{rt}_{u}")
                nc.vector.tensor_scalar(out=nmr, in0=mv[:, 0:1],
                                        scalar1=rstd, scalar2=-1.0,
                                        op0=ALU.mult, op1=ALU.mult)
                scales.append((rstd, nmr))
            for u in range(2):
                rstd, nmr = scales[u]
                nc.scalar.activation(out=zt[:, u, :], in_=xt[:, u, :],
                                     func=AF.Identity, bias=nmr, scale=rstd)
                r0 = rt * 256 + u * 128
                nc.sync.dma_start(out=zT[:, :, r0:r0 + 128], in_=zt[:, u, :],
                                  transpose=True)

        # ---- stage 2: QKV projections (all transposed layout, 1024-wide) --
        for chk in range(2):
            ns = slice(chk * 1024, (chk + 1) * 1024)
            for pi, (w_t, c_t, dest) in enumerate(((wq_t, cq_t, qT),
                                                   (wk_t, ck_t, None),
                                                   (wv_t, cv_t, vT))):
                for j in range(2):
                    pq = ps2.tile([128, 1024], f32, tag="pq", bufs=3,
                                  name=f"pq{chk}_{pi}_{j}")
                    for t in range(8):
                        lhs = w_t[:, t, j * 128:(j + 1) * 128]
                        for half in range(2):
                            hs = slice(half * 512, (half + 1) * 512)
                            nc.tensor.matmul(
                                pq[:, hs], lhs,
                                zT[:, t, chk * 1024 + half * 512:
                                   chk * 1024 + half * 512 + 512],
                                start=(t == 0), stop=(t == 7))
                    if pi == 0:
                        nc.scalar.activation(out=dest[:, j, ns], in_=pq,
                                             func=AF.Identity,
                                             bias=c_t[:, j:j + 1], scale=1.0)
                    elif pi == 1:
                        # k lands in the zero-padded kTz halves
                        for par in range(2):
                            ps = slice(par * 64, (par + 1) * 64)
                            nc.vector.tensor_scalar_add(
                                out=kTz[ps, 2 * j + par, ns], in0=pq[ps, :],
                                scalar1=c_t[ps, j:j + 1])
                    else:
                        eng = nc.scalar if j == 0 else nc.vector
                        if j == 0:
                            nc.scalar.activation(out=dest[:, j, ns], in_=pq,
                                                 func=AF.Identity,
                                                 bias=c_t[:, j:j + 1],
                                                 scale=1.0)
                        else:
                            nc.vector.tensor_scalar_add(
                                out=dest[:, j, ns], in0=pq,
                                scalar1=c_t[:, j:j + 1])

        # v: [vdim, n] -> [keys, kt, head, dim] via DMA-crossbar transpose.
        # The XBAR needs a 16B-aligned contiguous destination, so transpose
        # into a tmp tile and strided-copy into the 65-col augmented layout.
        for j in range(2):
            vtmp = vTp.tile([128, 16, 128], bf16, tag="vtmp", bufs=2,
                            name=f"vtmp{j}")
            nc.scalar.dma_start(out=vtmp, in_=vT[:, j, :], transpose=True)
            for hp in range(2):
                nc.scalar.dma_start(
                    out=vA4[:, :, 2 * j + hp, 0:DH],
                    in_=vtmp[:, :, 64 * hp:64 * hp + 64])

        ps2_cm.__exit__(None, None, None)
        stp_cm.__exit__(None, None, None)
        zp_cm.__exit__(None, None, None)
        vTp_cm.__exit__(None, None, None)
        zTp_cm.__exit__(None, None, None)
        wstage_cm.__exit__(None, None, None)
        xpre_cm.__exit__(None, None, None)

        # ---- stage 3: attention ----
        # Per head, the 2048 q columns are processed as four 512-col
        # quarters: even quarters get exact Exp on ScalarE, odd quarters the
        # DVE fast-exp (per-q engine consistency keeps the softmax
        # normalization exact for the approximation's common mode). Each
        # quarter has its own single-bank dots tile; AV runs one kt behind
        # the dots (software pipeline), so no dependency is tight and the PE
        # never drops out of its high-clock p-state. PSUM: 4 dots banks + 2
        # [65,1024] EV accumulators (4 banks) = 8.
        with tc.tile_pool(name="Ep", bufs=1) as Ep, \
             tc.tile_pool(name="rp", bufs=1) as rp, \
             tc.tile_pool(name="psD", bufs=1, space="PSUM") as psD, \
             tc.tile_pool(name="psU", bufs=1, space="PSUM") as psU:
            for h in range(HPC):
                j, p0 = h // 2, 64 * (h % 2)
                pU0 = psU.tile([DH + 1, 1024], f32, tag="pU0", name=f"pU0_{h}")
                pU1 = psU.tile([DH + 1, 1024], f32, tag="pU1", name=f"pU1_{h}")
                pUq = [pU0[:, 0:512], pU0[:, 512:1024],
                       pU1[:, 0:512], pU1[:, 512:1024]]
                vh = vA[:, :, h * (DH + 1):(h + 1) * (DH + 1)]
                prev = None
                for kt in range(16):
                    ksl = slice(kt * 128, (kt + 1) * 128)
                    lhs = kTz[:, h, ksl]
                    pDs, Ets = [], []
                    for qq in range(4):
                        pD = psD.tile([128, 512], f32, tag=f"pD{qq}",
                                      name=f"pD{qq}_{h}_{kt}")
                        nc.tensor.matmul(
                            pD, lhs, qT[:, j, qq * 512:(qq + 1) * 512],
                            start=True, stop=True)
                        pDs.append(pD)
                    for qq in range(4):
                        if qq % 2 == 0:
                            Et = Ep.tile([128, 512], bf16, tag=f"Et{qq}",
                                         bufs=2, name=f"Et{qq}_{h}_{kt}")
                            nc.scalar.activation(out=Et, in_=pDs[qq],
                                                 func=AF.Exp, bias=0.0,
                                                 scale=1.0)
                            Ets.append(Et)
                        else:
                            Et = Ep.tile([128, 512], i16, tag=f"Et{qq}",
                                         bufs=2, name=f"Et{qq}_{h}_{kt}")
                            nc.vector.tensor_scalar(out=Et, in0=pDs[qq],
                                                    scalar1=FE_S1,
                                                    scalar2=FE_S2,
                                                    op0=ALU.mult, op1=ALU.add)
                            Ets.append(Et.bitcast(bf16))
                    if prev is not None:
                        pkt, pEts = prev
                        for qq in range(4):
                            nc.tensor.matmul(pUq[qq], vh[:, pkt, :], pEts[qq],
                                             start=(pkt == 0),
                                             stop=(pkt == 15))
                    prev = (kt, Ets)
                pkt, pEts = prev
                for qq in range(4):
                    nc.tensor.matmul(pUq[qq], vh[:, pkt, :], pEts[qq],
                                     start=(pkt == 0), stop=(pkt == 15))
                # normalization: rowsum (row 64) -> DRAM -> [64,16] lanes ->
                # reciprocal -> DRAM -> partition-broadcast -> multiply
                for qh, pU in ((0, pU0), (1, pU1)):
                    slot = sum_d[h * 2 + qh]
                    uS = rp.tile([1, 1024], f32, tag="uS", bufs=2,
                                 name=f"uS_{h}_{qh}")
                    nc.scalar.copy(out=uS, in_=pU[DH:DH + 1, :])
                    nc.sync.dma_start(out=slot, in_=uS)
                    r8 = rp.tile([64, 16], f32, tag="r8", bufs=2,
                                 name=f"r8_{h}_{qh}")
                    nc.sync.dma_start(
                        out=r8, in_=slot.rearrange("(p e) -> p e", p=64))
                    nc.vector.reciprocal(out=r8, in_=r8)
                    rslot = rec_d[h * 2 + qh]
                    nc.sync.dma_start(out=rslot, in_=r8)
                    recB = rp.tile([64, 1024], f32, tag="recB", bufs=2,
                                   name=f"recB_{h}_{qh}")
                    rbc = bass.AP(tensor=rslot.tensor, offset=rslot.offset,
                                  ap=[[0, 64]] + list(rslot.ap))
                    nc.gpsimd.dma_start(out=recB, in_=rbc)
                    nc.vector.tensor_mul(
                        out=outT[p0:p0 + 64, j, qh * 1024:(qh + 1) * 1024],
                        in0=pU[0:DH, :], in1=recB)

        # ---- stage 4: output projection ----
        with tc.tile_pool(name="osb", bufs=4) as osb, \
             tc.tile_pool(name="psO", bufs=1, space="PSUM") as psO:
            out_r = out_d.rearrange("(m p) d -> m p d", p=128)
            for m in range(16):
                pO = psO.tile([128, 1024], f32, tag="pO", bufs=2,
                              name=f"pO{m}")
                for j in range(2):
                    lhs = outT[:, j, m * 128:(m + 1) * 128]
                    for half in range(2):
                        hs = slice(half * 512, (half + 1) * 512)
                        nc.tensor.matmul(pO[:, hs], lhs, wo_t[:, j, hs],
                                         start=(j == 0), stop=(j == 1))
                ot = osb.tile([128, 1024], f32, tag="ot", name=f"ot{m}")
                if m % 2 == 0:
                    nc.scalar.activation(out=ot, in_=pO, func=AF.Identity,
                                         bias=0.0, scale=1.0)
                else:
                    nc.vector.tensor_copy(out=ot, in_=pO)
                eng = nc.sync if m % 2 == 0 else nc.scalar
                eng.dma_start(out=out_r[m], in_=ot)

    nc.compile()
    return nc


def make_in_maps(x, ln_g, ln_b, Wq, Wkv, Wout):
    x = np.asarray(x, np.float32)
    ln_g = np.asarray(ln_g, np.float32)
    ln_b = np.asarray(ln_b, np.float32)
    Wq = np.asarray(Wq, np.float32)
    Wkv = np.asarray(Wkv, np.float32)
    Wout = np.asarray(Wout, np.float32)

    scale = DH ** -0.5
    Wq_f = (ln_g[:, None] * Wq) * scale
    cq_f = (ln_b @ Wq) * scale
    Wk_f = ln_g[:, None] * Wkv[:, :D]
    ck_f = ln_b @ Wkv[:, :D]
    Wv_f = ln_g[:, None] * Wkv[:, D:]
    cv_f = ln_b @ Wkv[:, D:]

    in_maps = []
    for c in range(N_CORES):
        cols = slice((c % 4) * CW, (c % 4 + 1) * CW)
        in_maps.append({
            "x": np.ascontiguousarray(x[c // 4]),
            "wq": np.ascontiguousarray(Wq_f[:, cols]),
            "wk": np.ascontiguousarray(Wk_f[:, cols]),
            "wv": np.ascontiguousarray(Wv_f[:, cols]),
            "wo": np.ascontiguousarray(Wout[cols, :]),
            "cq": np.ascontiguousarray(cq_f[cols]),
            "ck": np.ascontiguousarray(ck_f[cols]),
            "cv": np.ascontiguousarray(cv_f[cols]),
        })
    return in_maps


def kernel(x, ln_g, ln_b, Wq, Wkv, Wout):
    global _compiled, LAST_RESULT
    if _compiled is None:
        _compiled = _build()
    nc = _compiled

    in_maps = make_in_maps(x, ln_g, ln_b, Wq, Wkv, Wout)
    res = run_bass_kernel_spmd(nc, in_maps, list(range(N_CORES)), trace=TRACE)
    LAST_RESULT = res

    out = np.zeros((B, N, D), np.float32)
    for c in range(N_CORES):
        out[c // 4] += res.results[c]["out"]
    return out
